# revision 19
# baseline (speedup 1.0000x reference)
"""Trainium2 Bass kernel for nn_MultiHeadGAT (2-layer GAT + output heads).

Self-contained: accepts FULL inputs, shards across 8 NeuronCores internally,
returns the FULL output tuple (wire_logits, terminal_logits, action_logits).

Strategy (1D node partition by destination):
- Nodes sharded 2500/core (padded to 2560). Edges assigned to the core that
  owns their destination (col), sorted by col, grouped into 128-node chunks,
  padded to a uniform number of 128-edge blocks per chunk (NBC).
- Dense per-node work (x@W, attention dots, self-loop terms) is node-local.
- Per-layer node records [h(256) | alpha_src(4) | pad] are AllGathered so
  every core can gather any source node's record with gpsimd dma_gather.
- Edge softmax needs no max subtraction (exp args are O(1) for this data);
  normalization is folded to after aggregation: out = (sum ex*h) / (sum ex).
- Self-loops never enter the edge stream: handled densely per node using
  deg / sum(ae) collected as extra message columns in the layer-1 pass.
- Aggregation = PE matmul with a one-hot selection matrix per 128-edge block
  (S[p, n] = colrel[p] == n), accumulated in PSUM per 128-node chunk.
"""

import numpy as np
from contextlib import ExitStack

import concourse.bacc as bacc
import concourse.bass as bass
import concourse.mybir as mybir
import concourse.tile as tile
from concourse._compat import get_trn_type
from concourse.bass_utils import run_bass_kernel_spmd

FP = mybir.dt.float32
I16 = mybir.dt.int16
AF = mybir.ActivationFunctionType
OP = mybir.AluOpType

P = 128
HEADS, HID = 4, 64
HC = HEADS * HID  # 256


class Cfg:
    def __init__(self, n=20000, ncores=8, in_dim=128, sbb=8):
        self.N = n
        self.NCORES = ncores
        self.IN_DIM = in_dim
        self.NSH = n // ncores                      # real nodes per core
        self.NCH = (self.NSH + P - 1) // P          # chunks per core
        self.NSHP = self.NCH * P                    # padded nodes per core
        self.NTOT = self.NSHP * ncores              # padded global rows
        self.SBB = sbb                              # blocks per gather super-block
        self.RECW = 320                             # f32 record width (1280B)
        self.ADSTW = 64                             # f32 adst record width (256B)
        self.MSG1 = 269                             # h*ex(256) + ex(4) + ae1,ae2(8) + 1
        self.MSG2 = 260


def _fold_M(We, a_edge):
    # ae[e,h] = sum_c (eattr @ We)[e, h*64+c] * a_edge[h,c]  ==  eattr @ M
    return np.einsum('ihk,hk->ih', np.asarray(We, np.float32).reshape(2, HEADS, HID),
                     np.asarray(a_edge, np.float32))


def _wrap_idx(vals, nsb, per_sb):
    """dma_gather index layout: ordinal i of super-block g is stored at
    [i % 16, g*per_sb//16? ...]: per SB slab [16, per_sb//16], replicated to
    128 partitions; slabs concatenated along free dim."""
    cols = per_sb // 16
    out = np.zeros((128, nsb * cols), np.int16)
    for g in range(nsb):
        slab = vals[g * per_sb:(g + 1) * per_sb].reshape(cols, 16).T  # [16, cols]
        out[:, g * cols:(g + 1) * cols] = np.tile(slab, (8, 1))
    return out


def prep_host(inputs, cfg):
    """Index-only host prep: sort/shard edges, build per-core input maps."""
    c = cfg
    x = np.ascontiguousarray(np.asarray(inputs['x'], np.float32))
    eattr = np.asarray(inputs['edge_attr'], np.float32)
    ei = np.asarray(inputs['edge_index'])
    row = ei[0].astype(np.int64)
    col = ei[1].astype(np.int64)

    M1 = _fold_M(inputs['We1'], inputs['att_edge1'])
    M2 = _fold_M(inputs['We2'], inputs['att_edge2'])

    order = np.argsort(col, kind='stable')
    srow, scol, sattr = row[order], col[order], eattr[order]
    core_of = scol // c.NSH

    # per (core, chunk) edge lists
    per_core = []
    nbc = 0
    for cc in range(c.NCORES):
        m = core_of == cc
        crow, ccol, cat = srow[m], scol[m] - cc * c.NSH, sattr[m]
        chunks = []
        for k in range(c.NCH):
            mm = (ccol >= k * P) & (ccol < (k + 1) * P)
            chunks.append((crow[mm], (ccol[mm] - k * P), cat[mm]))
            nbc = max(nbc, (len(chunks[-1][0]) + P - 1) // P)
        per_core.append(chunks)
    nbc = max(2, nbc)
    while (c.NCH * nbc) % c.SBB:
        nbc += 1
    NB = c.NCH * nbc
    NSB = NB // c.SBB

    W1 = np.asarray(inputs['W1'], np.float32)
    W2 = np.asarray(inputs['W2'], np.float32)
    W2p = np.concatenate([W2[0:128, :], W2[128:256, :]], axis=1)  # [128, 512]
    Wcat = np.concatenate([np.asarray(inputs['Ww'], np.float32),
                           np.asarray(inputs['Wt'], np.float32),
                           np.asarray(inputs['Wa'], np.float32)], axis=1)  # [64,106]
    bcat = np.concatenate([np.asarray(inputs['bw'], np.float32),
                           np.asarray(inputs['bt'], np.float32),
                           np.asarray(inputs['ba'], np.float32)])[None, :]

    def bc(row):
        return np.ascontiguousarray(np.tile(np.asarray(row, np.float32).reshape(1, -1), (P, 1)))

    common = dict(
        W1=np.ascontiguousarray(W1),
        W2p=np.ascontiguousarray(W2p),
        Wcat=np.ascontiguousarray(Wcat),
        bcatv=bc(bcat),
        b1v=bc(inputs['b1']),
        b2v=bc(inputs['b2']),
        as1v=bc(np.asarray(inputs['att_src1'], np.float32).reshape(1, HC)),
        ad1v=bc(np.asarray(inputs['att_dst1'], np.float32).reshape(1, HC)),
        as2v=bc(np.asarray(inputs['att_src2'], np.float32).reshape(1, HC)),
        ad2v=bc(np.asarray(inputs['att_dst2'], np.float32).reshape(1, HC)),
        iotav=bc(np.arange(P, dtype=np.float32)),
        identv=np.eye(P, dtype=np.float32),
    )

    in_maps = []
    for cc in range(c.NCORES):
        rows_l, colrel_l, attr_l, colloc_l = [], [], [], []
        for k in range(c.NCH):
            r, cr, at = per_core[cc][k]
            npad = nbc * P - len(r)
            # gather-row remap: global node g -> (g // NSH) * NSHP + g % NSH
            rg = (r // c.NSH) * c.NSHP + (r % c.NSH)
            rows_l.append(np.concatenate([rg, np.zeros(npad, np.int64)]))
            colrel_l.append(np.concatenate([cr.astype(np.float32),
                                            -np.ones(npad, np.float32)]))
            colloc_l.append(np.concatenate([cr + k * P, np.zeros(npad, np.int64)]))
            attr_l.append(np.concatenate([at, np.zeros((npad, 2), np.float32)]))
        rows = np.concatenate(rows_l)
        colrel = np.concatenate(colrel_l)
        colloc = np.concatenate(colloc_l)
        attr = np.concatenate(attr_l, axis=0)

        xs = x[cc * c.NSH:(cc + 1) * c.NSH]
        xT = np.zeros((c.IN_DIM, c.NSHP), np.float32)
        xT[:, :c.NSH] = xs.T

        im = dict(common)
        im.update(
            xT=np.ascontiguousarray(xT),
            eattr_w=np.ascontiguousarray(attr.reshape(NB, P, 2).transpose(1, 0, 2)),
            colrel=np.ascontiguousarray(colrel.reshape(NB, P).T),
            rowidx=_wrap_idx(rows.astype(np.int16), NSB, c.SBB * P),
            colidx=_wrap_idx(colloc.astype(np.int16), NSB, c.SBB * P),
        )
        in_maps.append(im)

    return in_maps, nbc, M1, M2


def build_program(cfg, nbc, M1, M2):
    c = cfg
    NB = c.NCH * nbc
    NSB = NB // c.SBB
    SBB = c.SBB

    nc = bacc.Bacc(get_trn_type() or "TRN2", num_devices=c.NCORES)

    # --- I/O ---
    xT_d = nc.dram_tensor("xT", [c.IN_DIM, c.NSHP], FP, kind="ExternalInput")
    W1_d = nc.dram_tensor("W1", [c.IN_DIM, HC], FP, kind="ExternalInput")
    W2p_d = nc.dram_tensor("W2p", [P, 2 * HC], FP, kind="ExternalInput")
    Wcat_d = nc.dram_tensor("Wcat", [HID, 106], FP, kind="ExternalInput")
    bcat_d = nc.dram_tensor("bcatv", [P, 106], FP, kind="ExternalInput")
    b1_d = nc.dram_tensor("b1v", [P, HC], FP, kind="ExternalInput")
    b2_d = nc.dram_tensor("b2v", [P, HID], FP, kind="ExternalInput")
    as1_d = nc.dram_tensor("as1v", [P, HC], FP, kind="ExternalInput")
    ad1_d = nc.dram_tensor("ad1v", [P, HC], FP, kind="ExternalInput")
    as2_d = nc.dram_tensor("as2v", [P, HC], FP, kind="ExternalInput")
    ad2_d = nc.dram_tensor("ad2v", [P, HC], FP, kind="ExternalInput")
    iota_d = nc.dram_tensor("iotav", [P, P], FP, kind="ExternalInput")
    ident_d = nc.dram_tensor("identv", [P, P], FP, kind="ExternalInput")
    eattr_d = nc.dram_tensor("eattr_w", [P, NB, 2], FP, kind="ExternalInput")
    colrel_d = nc.dram_tensor("colrel", [P, NB], FP, kind="ExternalInput")
    rowidx_d = nc.dram_tensor("rowidx", [P, NB * 8], I16, kind="ExternalInput")
    colidx_d = nc.dram_tensor("colidx", [P, NB * 8], I16, kind="ExternalInput")
    logits_d = nc.dram_tensor("logits", [c.NSHP, 106], FP, kind="ExternalOutput")

    rg = [list(range(c.NCORES))]

    with tile.TileContext(nc) as tc, ExitStack() as stk:
        dramp = stk.enter_context(tc.tile_pool(name="dram", bufs=1, space="DRAM"))
        pers = stk.enter_context(tc.tile_pool(name="pers", bufs=1))
        work = stk.enter_context(tc.tile_pool(name="work", bufs=2))
        psA = stk.enter_context(tc.tile_pool(name="psA", bufs=2, space="PSUM"))
        psAgg = stk.enter_context(tc.tile_pool(name="psAgg", bufs=2, space="PSUM"))
        psT = stk.enter_context(tc.tile_pool(name="psT", bufs=2, space="PSUM"))

        recsh1 = dramp.tile([c.NSHP, c.RECW], FP)
        recfull1 = dramp.tile([c.NTOT, c.RECW], FP, addr_space="Shared")
        recsh2 = dramp.tile([c.NSHP, c.RECW], FP)
        recfull2 = dramp.tile([c.NTOT, c.RECW], FP, addr_space="Shared")
        adst1_t = dramp.tile([c.NSHP, c.ADSTW], FP)
        adst2_t = dramp.tile([c.NSHP, c.ADSTW], FP)

        # --- constants to SBUF ---
        def bload(dram_t, parts, width, name):
            t = pers.tile([parts, width], FP, name=name)
            nc.sync.dma_start(t[:], dram_t[:, :])
            return t

        W1s = pers.tile([c.IN_DIM, HC], FP)
        nc.sync.dma_start(W1s[:], W1_d[:, :])
        W2s = pers.tile([P, 2 * HC], FP)
        nc.sync.dma_start(W2s[:], W2p_d[:, :])
        Wcats = pers.tile([HID, 106], FP)
        nc.sync.dma_start(Wcats[:], Wcat_d[:, :])
        idents = pers.tile([P, P], FP)
        nc.sync.dma_start(idents[:], ident_d[:, :])

        b1s = bload(b1_d, P, HC, "b1s")
        b2s = bload(b2_d, P, HID, "b2s")
        bcats = bload(bcat_d, P, 106, "bcats")
        as1s = bload(as1_d, P, HC, "as1s")
        ad1s = bload(ad1_d, P, HC, "ad1s")
        as2s = bload(as2_d, P, HC, "as2s")
        ad2s = bload(ad2_d, P, HC, "ad2s")
        iotas = bload(iota_d, P, P, "iotas")

        colrels = pers.tile([P, NB], FP)
        nc.sync.dma_start(colrels[:], colrel_d[:, :])
        ae9s = pers.tile([P, NB, 9], FP)

        # --- persistent per-node state ---
        h1s = pers.tile([P, c.NCH, HC], FP)
        h2s = pers.tile([P, c.NCH, HC], FP)
        h2eT = pers.tile([P, 2, c.NCH, P], FP)
        HTs = pers.tile([HID, c.NCH, P], FP)
        asrc1s = pers.tile([P, c.NCH, HEADS], FP)
        adst1s = pers.tile([P, c.NCH, HEADS], FP)
        asrc2s = pers.tile([P, c.NCH, HEADS], FP)
        adst2s = pers.tile([P, c.NCH, HEADS], FP)
        aeL2s = pers.tile([P, c.NCH, HEADS], FP)

        def dense_node_phase(layer):
            """h = (input) @ W; asrc/adst; write records + adst table."""
            h_s = h1s if layer == 1 else h2s
            asrc_s, adst_s = (asrc1s, adst1s) if layer == 1 else (asrc2s, adst2s)
            a_s, d_s = (as1s, ad1s) if layer == 1 else (as2s, ad2s)
            recsh = recsh1 if layer == 1 else recsh2
            adst_t = adst1_t if layer == 1 else adst2_t
            for k in range(c.NCH):
                ph = psA.tile([P, HC], FP, tag="ph")
                if layer == 1:
                    nc.tensor.matmul(out=ph[:], lhsT=xTs[:, k * P:(k + 1) * P],
                                     rhs=W1s[:], start=True, stop=True)
                else:
                    for kt in range(2):
                        nc.tensor.matmul(out=ph[:], lhsT=h2eT[:, kt, k, :],
                                         rhs=W2s[:, kt * HC:(kt + 1) * HC],
                                         start=(kt == 0), stop=(kt == 1))
                nc.vector.tensor_copy(out=h_s[:, k, :], in_=ph[:])
                tm = work.tile([P, HC], FP, tag="tmul")
                nc.vector.tensor_tensor(out=tm[:], in0=h_s[:, k, :], in1=a_s[:], op=OP.mult)
                nc.vector.tensor_reduce(out=asrc_s[:, k, :],
                                        in_=tm[:].rearrange("p (h c) -> p h c", h=HEADS),
                                        axis=mybir.AxisListType.X, op=OP.add)
                nc.vector.tensor_tensor(out=tm[:], in0=h_s[:, k, :], in1=d_s[:], op=OP.mult)
                nc.vector.tensor_reduce(out=adst_s[:, k, :],
                                        in_=tm[:].rearrange("p (h c) -> p h c", h=HEADS),
                                        axis=mybir.AxisListType.X, op=OP.add)
                rows = slice(k * P, (k + 1) * P)
                nc.sync.dma_start(recsh[rows, 0:HC], h_s[:, k, :])
                nc.sync.dma_start(recsh[rows, HC:HC + HEADS], asrc_s[:, k, :])
                nc.sync.dma_start(adst_t[rows, 0:HEADS], adst_s[:, k, :])

        def merge_chunk(layer, k, ps):
            """Self-loop merge + normalize for chunk k from PSUM agg tile."""
            h_s = h1s if layer == 1 else h2s
            asrc_s, adst_s = (asrc1s, adst1s) if layer == 1 else (asrc2s, adst2s)
            exl = work.tile([P, HEADS], FP, tag="exl")
            t4a = work.tile([P, HEADS], FP, tag="t4a")
            t4b = work.tile([P, HEADS], FP, tag="t4b")
            if layer == 1:
                degm = work.tile([P, 1], FP, tag="degm")
                rdeg = work.tile([P, 1], FP, tag="rdeg")
                nc.vector.tensor_scalar_max(degm[:], ps[:, 268:269], 1.0)
                nc.vector.reciprocal(rdeg[:], degm[:])
                nc.vector.tensor_scalar(out=t4a[:], in0=ps[:, 260:264],
                                        scalar1=rdeg[:, :], scalar2=None, op0=OP.mult)
                nc.vector.tensor_scalar(out=aeL2s[:, k, :], in0=ps[:, 264:268],
                                        scalar1=rdeg[:, :], scalar2=None, op0=OP.mult)
            nc.vector.tensor_tensor(out=t4b[:], in0=asrc_s[:, k, :],
                                    in1=adst_s[:, k, :], op=OP.add)
            nc.vector.tensor_tensor(out=t4b[:], in0=t4b[:],
                                    in1=(t4a[:] if layer == 1 else aeL2s[:, k, :]),
                                    op=OP.add)
            nc.vector.tensor_scalar_mul(t4a[:], t4b[:], 0.2)
            nc.vector.tensor_tensor(out=t4b[:], in0=t4b[:], in1=t4a[:], op=OP.max)
            nc.scalar.activation(out=exl[:], in_=t4b[:], func=AF.Exp)
            den = work.tile([P, HEADS], FP, tag="den")
            rden = work.tile([P, HEADS], FP, tag="rden")
            nc.vector.tensor_tensor(out=den[:], in0=ps[:, HC:HC + HEADS], in1=exl[:], op=OP.add)
            if layer == 2:
                nc.vector.tensor_scalar_mul(den[:], den[:], 4.0)
            nc.vector.reciprocal(rden[:], den[:])

            o = work.tile([P, HC], FP, tag="o")
            sh = work.tile([P, HID], FP, tag="sh")
            for h in range(HEADS):
                cols = slice(h * HID, (h + 1) * HID)
                nc.scalar.activation(out=sh[:], in_=h_s[:, k, cols], func=AF.Copy,
                                     scale=exl[:, h:h + 1])
                nc.vector.tensor_tensor(out=sh[:], in0=sh[:], in1=ps[:, cols], op=OP.add)
                nc.scalar.activation(out=o[:, cols], in_=sh[:], func=AF.Copy,
                                     scale=rden[:, h:h + 1])
            if layer == 1:
                g = work.tile([P, HC], FP, tag="g")
                nc.vector.tensor_tensor(out=g[:], in0=o[:], in1=b1s[:], op=OP.add)
                helu = work.tile([P, HC], FP, tag="helu")
                _elu(g, helu, HC)
                for kt in range(2):
                    pt = psT.tile([P, P], FP, tag="pt")
                    nc.tensor.transpose(out=pt[:], in_=helu[:, kt * P:(kt + 1) * P],
                                        identity=idents[:])
                    nc.vector.tensor_copy(out=h2eT[:, kt, k, :], in_=pt[:])
            else:
                om = work.tile([P, HID], FP, tag="om")
                nc.vector.tensor_tensor(out=om[:], in0=o[:, 0:HID], in1=o[:, HID:2 * HID], op=OP.add)
                nc.vector.tensor_tensor(out=om[:], in0=om[:], in1=o[:, 2 * HID:3 * HID], op=OP.add)
                nc.vector.tensor_tensor(out=om[:], in0=om[:], in1=o[:, 3 * HID:4 * HID], op=OP.add)
                g = work.tile([P, HID], FP, tag="g2")
                nc.vector.tensor_tensor(out=g[:], in0=om[:], in1=b2s[:], op=OP.add)
                hf = work.tile([P, HID], FP, tag="hf")
                _elu(g, hf, HID)
                pt = psT.tile([P, P], FP, tag="pt", name="ptH")
                nc.tensor.transpose(out=pt[0:HID, 0:P], in_=hf[:], identity=idents[:])
                nc.vector.tensor_copy(out=HTs[:, k, :], in_=pt[0:HID, 0:P])

        def _elu(g, out, w):
            mneg = work.tile([P, w], FP, tag=f"mneg{w}")
            epos = work.tile([P, w], FP, tag=f"epos{w}")
            nc.vector.tensor_scalar_min(mneg[:], g[:], 0.0)
            nc.scalar.activation(out=epos[:], in_=mneg[:], func=AF.Exp)
            nc.vector.tensor_scalar_max(mneg[:], g[:], 0.0)
            nc.vector.tensor_tensor(out=epos[:], in0=epos[:], in1=mneg[:], op=OP.add)
            nc.vector.tensor_scalar_add(out[:], epos[:], -1.0)

        def edge_pass(layer):
            recfull = recfull1 if layer == 1 else recfull2
            adst_t = adst1_t if layer == 1 else adst2_t
            msgw = c.MSG1 if layer == 1 else c.MSG2
            aeoff = 0 if layer == 1 else 4
            cur = {}
            for sb in range(NSB):
                icols = slice(sb * SBB * 8, (sb + 1) * SBB * 8)
                ridx = work.tile([P, SBB * 8], I16, tag="ridx")
                nc.sync.dma_start(ridx[:], rowidx_d[:, icols])
                cidx = work.tile([P, SBB * 8], I16, tag="cidx")
                nc.sync.dma_start(cidx[:], colidx_d[:, icols])
                grec = grecp.tile([P, SBB, c.RECW], FP, tag="grec")
                nc.gpsimd.dma_gather(grec[:], recfull[:, :], ridx[:],
                                     SBB * P, SBB * P, c.RECW)
                gadst = grecp.tile([P, SBB, c.ADSTW], FP, tag="gadst")
                nc.gpsimd.dma_gather(gadst[:], adst_t[:, :], cidx[:],
                                     SBB * P, SBB * P, c.ADSTW)
                sbsl = slice(sb * SBB, (sb + 1) * SBB)
                al = msgp.tile([P, SBB, HEADS], FP, tag="al")
                ex = msgp.tile([P, SBB, HEADS], FP, tag="ex")
                nc.vector.tensor_tensor(out=al[:], in0=grec[:, :, HC:HC + HEADS],
                                        in1=gadst[:, :, 0:HEADS], op=OP.add)
                nc.vector.tensor_tensor(out=al[:], in0=al[:],
                                        in1=ae9s[:, sbsl, aeoff:aeoff + 4], op=OP.add)
                nc.vector.tensor_scalar_mul(ex[:], al[:], 0.2)
                nc.vector.tensor_tensor(out=al[:], in0=al[:], in1=ex[:], op=OP.max)
                nc.scalar.activation(out=ex[:], in_=al[:], func=AF.Exp)
                msg = msgp.tile([P, SBB, msgw], FP, tag="msg")
                nc.vector.tensor_tensor(
                    out=msg[:, :, 0:HC].rearrange("p s (h c) -> p s h c", h=HEADS),
                    in0=grec[:, :, 0:HC].rearrange("p s (h c) -> p s h c", h=HEADS),
                    in1=ex[:].to_broadcast([P, SBB, HEADS, HID]), op=OP.mult)
                nc.vector.tensor_copy(out=msg[:, :, HC:HC + HEADS], in_=ex[:])
                if layer == 1:
                    nc.vector.tensor_copy(out=msg[:, :, HC + HEADS:c.MSG1],
                                          in_=ae9s[:, sbsl, :])
                for b in range(SBB):
                    blk = sb * SBB + b
                    k, i = blk // nbc, blk % nbc
                    if i == 0:
                        cur['ps'] = psAgg.tile([P, msgw], FP, tag="agg",
                                               name=f"agg_{layer}_{blk}")
                    S = work.tile([P, P], FP, tag="S")
                    nc.vector.tensor_tensor(
                        out=S[:], in0=iotas[:],
                        in1=colrels[:, blk:blk + 1].to_broadcast([P, P]),
                        op=OP.is_equal)
                    nc.tensor.matmul(out=cur['ps'][:], lhsT=S[:], rhs=msg[:, b, :],
                                     start=(i == 0), stop=(i == nbc - 1))
                    if i == nbc - 1:
                        merge_chunk(layer, k, cur['ps'])

        # ---- pipeline ----
        with tc.tile_pool(name="tmp0", bufs=1) as tmp0:
            xTs = tmp0.tile([c.IN_DIM, c.NSHP], FP)
            nc.sync.dma_start(xTs[:], xT_d[:, :])
            eattrs = tmp0.tile([P, NB, 2], FP)
            nc.sync.dma_start(eattrs[:], eattr_d[:, :])
            tmpnb = tmp0.tile([P, NB], FP)
            # ae9: [ae1(4) | ae2(4) | 1] per edge
            for li, M in ((0, M1), (1, M2)):
                for h in range(HEADS):
                    cidx = li * HEADS + h
                    nc.vector.tensor_scalar_mul(ae9s[:, :, cidx], eattrs[:, :, 0],
                                                float(M[0, h]))
                    nc.vector.tensor_scalar_mul(tmpnb[:], eattrs[:, :, 1],
                                                float(M[1, h]))
                    nc.vector.tensor_tensor(out=ae9s[:, :, cidx],
                                            in0=ae9s[:, :, cidx],
                                            in1=tmpnb[:], op=OP.add)
            nc.gpsimd.memset(ae9s[:, :, 8], 1.0)
            dense_node_phase(1)
        nc.gpsimd.collective_compute("AllGather", OP.bypass, replica_groups=rg,
                                     ins=[recsh1.opt()], outs=[recfull1.opt()])
        with tc.tile_pool(name="grec", bufs=2) as grecp, \
             tc.tile_pool(name="msg", bufs=2) as msgp:
            edge_pass(1)
            dense_node_phase(2)
            nc.gpsimd.collective_compute("AllGather", OP.bypass, replica_groups=rg,
                                         ins=[recsh2.opt()], outs=[recfull2.opt()])
            edge_pass(2)

        # ---- output heads ----
        for k in range(c.NCH):
            pl = psA.tile([P, 106], FP, tag="ph", name="pl")
            nc.tensor.matmul(out=pl[:], lhsT=HTs[:, k, :], rhs=Wcats[:],
                             start=True, stop=True)
            lo = work.tile([P, 106], FP, tag="lo")
            nc.vector.tensor_tensor(out=lo[:], in0=pl[:], in1=bcats[:], op=OP.add)
            nc.sync.dma_start(logits_d[k * P:(k + 1) * P, :], lo[:])

    nc.compile()
    return nc


_CACHE = {}
LAST_RESULTS = None


def _run(inputs, cfg, trace=False):
    global LAST_RESULTS
    in_maps, nbc, M1, M2 = prep_host(inputs, cfg)
    key = (cfg.N, cfg.NCORES, nbc, M1.tobytes(), M2.tobytes())
    if key not in _CACHE:
        _CACHE.clear()
        _CACHE[key] = build_program(cfg, nbc, M1, M2)
    nc = _CACHE[key]
    res = run_bass_kernel_spmd(nc, in_maps, list(range(cfg.NCORES)), trace=trace)
    LAST_RESULTS = res
    logits = np.concatenate(
        [res.results[i]['logits'][:cfg.NSH] for i in range(cfg.NCORES)], axis=0)
    nw = int(np.asarray(inputs['num_wires']))
    nt = int(np.asarray(inputs['num_terminals']))
    return (np.ascontiguousarray(logits[:, :nw]),
            np.ascontiguousarray(logits[:, 64:64 + nt]),
            np.ascontiguousarray(logits[:, 96:106]))


def kernel(**inputs):
    return _run(inputs, Cfg())
